# revision 61
# baseline (speedup 1.0000x reference)
"""Trainium2 Bass kernel: LocalWLGNN 3-hop GNN message passing on 8 NeuronCores.

Strategy (dst-node sharding):
  - out = (1+eps)*x + sum_h h_h, with per-hop recurrence
        h_new = a1 * G + w (.) x + c1,   G[r] = sum_{e: ni[e]=r} h[si[e]]
    where a1=(1+b1)(1+b3), c1=(1+b3), w=(1+b3)(deg + untouched + b2) are
    host-folded from the runtime scalar inputs (exact algebra, verified).
  - Nodes are dst-sharded across 8 cores (12500 rows each, padded to 12544).
    Each core computes G for its shard via dma_gather (random source rows,
    bf16, 4 SWDGE queues) + selection-matrix matmuls into PSUM block
    accumulators (128 dst rows per block), then the fused elementwise.
  - h is replicated: after hops 0,1 an AllGather shares the bf16 h shards,
    split into A/B halves (blocks 0-48 / 49-97) so the A half overlaps with
    the tail compute of its hop. The padded gather row space is
    [A halves of all cores | B halves] so each half is contiguous.
  - dma_gather indices are int16, so sources live in 4 source-range buckets
    of NPAD/4 (< 32768) rows each. One gather call is capped at 1024
    descriptors (64/engine, one 16KB fused packet) - hardware limit.
  - Schedule: per (bucket, anchor-group of 16 dst blocks), each core's edges
    (sorted by dst block) pack contiguously; the group is padded to the max
    count over cores rounded to 128. Tiles of 128 gather slots may span
    block boundaries and per-core run boundaries differ; each (tile, block)
    pair gets its own selection column where foreign slots carry off=200 so
    is_equal(off, iota) zeroes them. Pad slots gather bucket row 0.
"""

import sys

sys.path.insert(0, "/opt/trn_rl_repo")

import numpy as np
import ml_dtypes

from concourse import bass, bacc, mybir
import concourse.tile as tile
from concourse.bass_utils import run_bass_kernel_spmd

P = 128
D = 128
HOPS = 3

FULL_CFG = dict(
    N=100000,
    NCORES=8,
    S=12500,        # rows per core
    NB=98,          # dst blocks per core (ceil(S/128)); SPAD = NB*128 = 12544
    AB=8,           # dst blocks per anchor group (padding re-sync boundary)
    NBUCKETS=4,     # source-range buckets (NPAD/NBUCKETS must be < 32768)
)


def _derived(cfg):
    S, NB, NC = cfg["S"], cfg["NB"], cfg["NCORES"]
    SPAD = NB * P
    NPAD = NC * SPAD
    BUCKET = NPAD // cfg["NBUCKETS"]
    assert NPAD % cfg["NBUCKETS"] == 0 and BUCKET % 2 == 0 and BUCKET <= 32767
    return SPAD, NPAD, BUCKET


def _wrap_idx(a):
    """Gather-index SBUF layout: logical position i -> [partition i%16, free i//16],
    replicated across the 8 q7 cores (x8 on partitions)."""
    assert len(a) % 128 == 0
    w = a.reshape(-1, 16).T.astype(np.int16)
    return np.tile(w, (8, 1))


def _roundup(x, m):
    return (int(x) + m - 1) // m * m


def _plan_hop(ni_loc_list, si_pad_list, cfg):
    """Anchor-group schedule, uniform across cores.

    Returns (sched, percore):
      sched = dict(
        tot_idx, n_cols,
        bucket_start = [NBUK], n_chunks = [NBUK],   # 1024-slot gather chunks
        plan = [ per block j: list of (b, k_tile, col) in matmul order ])
      percore[c] = dict(idx=[128, tot_idx//16] int16, off=[128, n_cols] bf16)
    """
    NB, NBUK, AB, NC = cfg["NB"], cfg["NBUCKETS"], cfg["AB"], cfg["NCORES"]
    _, _, BUCKET = _derived(cfg)
    NG = (NB + AB - 1) // AB

    # sort each core's edges by (bucket, dst local row)
    counts = np.zeros((NC, NBUK, NB), np.int64)
    sorted_off = []   # dst offset within block, per core (sorted order)
    sorted_src = []   # source row relative to its bucket, per core
    for c in range(NC):
        ni, si = ni_loc_list[c], si_pad_list[c]
        buk = si // BUCKET
        key = buk * (1 << 15) + ni
        order = np.argsort(key, kind="stable")
        sorted_src.append((si - buk * BUCKET)[order].astype(np.int16))
        sorted_off.append((ni & 127).astype(np.float32)[order])
        cnt = np.zeros((NBUK, NB), np.int64)
        np.add.at(cnt, (buk, ni >> 7), 1)
        counts[c] = cnt

    # group sizes (max over cores, rounded to whole tiles)
    grp_cnt = counts.reshape(NC, NBUK, NG, -1).sum(axis=3) if NB % AB == 0 else \
        np.stack([counts[:, :, g * AB:(g + 1) * AB].sum(axis=2)
                  for g in range(NG)], axis=2)
    GSLOT = np.array([[_roundup(grp_cnt[:, b, g].max(), 128)
                       for g in range(NG)] for b in range(NBUK)], np.int64)

    # stream layout: per bucket, groups then round the bucket up to 1024
    goff = np.zeros((NBUK, NG), np.int64)
    bucket_start = np.zeros(NBUK, np.int64)
    bucket_len = np.zeros(NBUK, np.int64)
    pos = 0
    for b in range(NBUK):
        bucket_start[b] = pos
        p = 0
        for g in range(NG):
            goff[b, g] = p
            p += GSLOT[b, g]
        bucket_len[b] = _roundup(p, 1024)
        pos += bucket_len[b]
    tot_idx = pos
    n_chunks = (bucket_len // 1024).astype(np.int64)

    # per-(core, bucket, block) stream ranges
    blk_start = np.zeros((NC, NBUK, NB), np.int64)
    for b in range(NBUK):
        for g in range(NG):
            j0, j1 = g * AB, min((g + 1) * AB, NB)
            cum = np.cumsum(counts[:, b, j0:j1], axis=1)
            blk_start[:, b, j0:j1] = goff[b, g] + cum - counts[:, b, j0:j1]
    blk_end = blk_start + counts  # [NC, NBUK, NB]

    # tile range per (b, j): union over cores with edges
    have = counts > 0
    k_lo = np.where(have, blk_start >> 7, np.int64(1 << 40)).min(axis=0)
    k_hi = np.where(have, (blk_end - 1) >> 7, np.int64(-1)).max(axis=0)
    # [NBUK, NB]; k_hi < k_lo means no edges anywhere

    # matmul plan + column ids in (j, b, k) order
    plan = []
    col_arr = {}  # (b, k, j) -> col
    n_cols = 0
    for j in range(NB):
        mj = []
        for b in range(NBUK):
            if k_hi[b, j] < k_lo[b, j]:
                continue
            for k in range(int(k_lo[b, j]), int(k_hi[b, j]) + 1):
                col_arr[(b, k, j)] = n_cols
                mj.append((b, k, n_cols))
                n_cols += 1
        plan.append(mj)

    # per-core idx / off data
    percore = []
    for c in range(NC):
        idx_stream = np.zeros(tot_idx, np.int16)
        off_flat = np.full((n_cols, 128), 200.0, np.float32)
        epos = 0  # position in this core's sorted edge arrays
        for b in range(NBUK):
            for g in range(NG):
                j0, j1 = g * AB, min((g + 1) * AB, NB)
                n = int(grp_cnt[c, b, g])
                if n == 0:
                    continue
                base = bucket_start[b] + goff[b, g]
                src = sorted_src[c][epos:epos + n]
                off = sorted_off[c][epos:epos + n]
                epos += n
                idx_stream[base:base + n] = src
                # column of each edge: stream pos -> tile k -> (b,k,j)
                p = np.arange(n) + int(goff[b, g])
                ks = p >> 7
                js = j0 + np.searchsorted(
                    np.cumsum(counts[c, b, j0:j1]), np.arange(n), side="right")
                cols = np.fromiter(
                    (col_arr[(b, int(k), int(jj))] for k, jj in zip(ks, js)),
                    np.int64, n)
                off_flat[cols, p & 127] = off
        percore.append(dict(
            idx=_wrap_idx(idx_stream),
            off=np.ascontiguousarray(off_flat.T).astype(ml_dtypes.bfloat16)))

    sched = dict(tot_idx=int(tot_idx), n_cols=int(n_cols),
                 bucket_start=[int(x) for x in bucket_start],
                 n_chunks=[int(x) for x in n_chunks],
                 k_hi=k_hi, k_lo=k_lo, plan=plan)
    return sched, percore


def _build_nc(cfg, scheds, scalars, hops=HOPS):
    """Build the SPMD bass program. scheds: per-hop schedule; scalars: dict with
    eps, a1[h], c1[h] floats baked as immediates."""
    NB, NBUK, NC = cfg["NB"], cfg["NBUCKETS"], cfg["NCORES"]
    SPAD, NPAD, BUCKET = _derived(cfg)
    f32, bf16, i16 = mybir.dt.float32, mybir.dt.bfloat16, mybir.dt.int16
    AOP = mybir.AluOpType
    MW = 16  # max selection columns built per is_equal

    nc = bacc.Bacc("TRN2", target_bir_lowering=False, debug=False, num_devices=NC,
                   num_swdge_queues=4)

    HALF = (NB // 2) * P  # rows per A/B half of a core's shard

    xg = nc.dram_tensor("xg", [NPAD, D], bf16, kind="ExternalInput")
    wx1_d = [nc.dram_tensor(f"wx1_{h}", [P, NB * D], bf16, kind="ExternalInput")
             for h in range(HOPS)]
    oini_d = nc.dram_tensor("oini", [P, NB * D], bf16, kind="ExternalInput")
    iota_d = nc.dram_tensor("iota", [P, MW, P], bf16, kind="ExternalInput")
    idx_d = [nc.dram_tensor(f"idx{h}", [P, scheds[h]["tot_idx"] // 16], i16,
                            kind="ExternalInput") for h in range(HOPS)]
    off_d = [nc.dram_tensor(f"off{h}", [P, scheds[h]["n_cols"]], bf16,
                            kind="ExternalInput") for h in range(HOPS)]
    out_d = nc.dram_tensor("out", [P, NB * D], bf16, kind="ExternalOutput")

    a1 = scalars["a1"]

    with tile.TileContext(nc) as tc:
        with (
            tc.tile_pool(name="const", bufs=1) as cpool,
            tc.tile_pool(name="io", bufs=2) as iopool,
            tc.tile_pool(name="v", bufs=4) as vpool,
            tc.tile_pool(name="m", bufs=6) as mpool,
            tc.tile_pool(name="fin", bufs=4) as fpool,
            tc.tile_pool(name="ps", bufs=8, space="PSUM") as pspool,
            tc.tile_pool(name="dram", bufs=1, space="DRAM") as dpool,
        ):
            iota_t = cpool.tile([P, MW, P], bf16, name="iota_t")
            nc.sync.dma_start(out=iota_t[:], in_=iota_d[:])
            out_acc = cpool.tile([P, NB * D], bf16, name="out_acc")
            nc.sync.dma_start(out=out_acc[:], in_=oini_d[:])

            h_myA = [dpool.tile([HALF, D], bf16, name=f"h_myA{h}")
                     for h in range(HOPS - 1)]
            h_myB = [dpool.tile([SPAD - HALF, D], bf16, name=f"h_myB{h}")
                     for h in range(HOPS - 1)]
            h_fullA = [dpool.tile([NC * HALF, D], bf16, addr_space="Shared",
                                  name=f"h_fullA{h}") for h in range(HOPS - 1)]
            h_fullB = [dpool.tile([NPAD - NC * HALF, D], bf16, addr_space="Shared",
                                  name=f"h_fullB{h}") for h in range(HOPS - 1)]

            NBA = NC * HALF // BUCKET  # buckets in the A region

            def _table(hop, b):
                if hop == 0:
                    return xg, b * BUCKET
                if b < NBA:
                    return h_fullA[hop - 1], b * BUCKET
                return h_fullB[hop - 1], (b - NBA) * BUCKET

            for hop in range(hops):
                sched = scheds[hop]
                idx_t = iopool.tile([P, sched["tot_idx"] // 16], i16, tag="idx")
                nc.sync.dma_start(out=idx_t[:], in_=idx_d[hop][:])
                off_t = iopool.tile([P, sched["n_cols"]], bf16, tag="off")
                nc.sync.dma_start(out=off_t[:], in_=off_d[hop][:])
                wx1 = iopool.tile([P, NB * D], bf16, tag="wx1")
                nc.sync.dma_start(out=wx1[:], in_=wx1_d[hop][:])

                vt_chunks = {}
                issued = [0] * NBUK
                qctr = [0]

                def _ensure(b, cmax):
                    cmax = min(cmax, sched["n_chunks"][b] - 1)
                    while issued[b] <= cmax:
                        c = issued[b]
                        tbl, boff = _table(hop, b)
                        vt = vpool.tile([P, 8, D], bf16, tag=f"v{b}")
                        o = sched["bucket_start"][b] + c * 1024
                        nc.gpsimd.dma_gather(
                            vt[:],
                            tbl[boff:boff + BUCKET, :],
                            idx_t[:, o // 16:(o + 1024) // 16],
                            1024, 1024, D,
                            queue_num=qctr[0] % 4,
                        )
                        qctr[0] += 1
                        vt_chunks[(b, c)] = vt
                        issued[b] += 1

                for jg in range(NB):
                    mms = sched["plan"][jg]
                    for b in range(NBUK):
                        if sched["k_hi"][b, jg] >= sched["k_lo"][b, jg]:
                            _ensure(b, int(sched["k_hi"][b, jg]) // 8 + 1)
                    ps = None
                    if mms:
                        col0 = mms[0][2]
                        nmm = len(mms)
                        Ms = []
                        for mo in range(0, nmm, MW):
                            w = min(MW, nmm - mo)
                            M = mpool.tile([P, MW, P], bf16, name="M")
                            nc.vector.tensor_tensor(
                                out=M[:, :w, :],
                                in0=off_t[:, col0 + mo:col0 + mo + w]
                                    .to_broadcast([P, w, P]),
                                in1=iota_t[:, :w, :],
                                op=AOP.is_equal,
                            )
                            Ms.append(M)
                        ps = pspool.tile([P, D], f32, name="ps")
                        for i, (b, k, col) in enumerate(mms):
                            ci = col - col0
                            nc.tensor.matmul(
                                out=ps[:],
                                lhsT=Ms[ci // MW][:, ci % MW, :],
                                rhs=vt_chunks[(b, k // 8)][:, k % 8, :],
                                start=(i == 0),
                                stop=(i == nmm - 1),
                            )
                    wblk = wx1[:, jg * D:(jg + 1) * D]
                    hf2 = fpool.tile([P, D], bf16, name="hf2")
                    if mms:
                        if a1[hop] != 1.0:
                            nc.vector.tensor_scalar(
                                out=ps[:], in0=ps[:], scalar1=float(a1[hop]),
                                scalar2=None, op0=AOP.mult,
                            )
                        nc.vector.tensor_tensor(
                            out=hf2[:], in0=ps[:], in1=wblk, op=AOP.add)
                    else:
                        nc.vector.tensor_copy(out=hf2[:], in_=wblk)
                    oblk = out_acc[:, jg * D:(jg + 1) * D]
                    nc.vector.tensor_tensor(
                        out=oblk, in0=oblk, in1=hf2[:], op=AOP.add)
                    if hop < hops - 1:
                        if jg < NB // 2:
                            nc.sync.dma_start(
                                out=h_myA[hop][jg * P:(jg + 1) * P, :],
                                in_=hf2[:])
                        else:
                            j2 = jg - NB // 2
                            nc.sync.dma_start(
                                out=h_myB[hop][j2 * P:(j2 + 1) * P, :],
                                in_=hf2[:])
                    # fire the A-half AllGather once the A blocks (0-48) are
                    # done, a few blocks later so its wait is already satisfied
                    if hop < hops - 1 and jg == 70:
                        nc.gpsimd.collective_compute(
                            "AllGather",
                            mybir.AluOpType.bypass,
                            replica_groups=[list(range(NC))],
                            ins=[h_myA[hop].opt()],
                            outs=[h_fullA[hop].opt()],
                        )
                if hop < hops - 1:
                    nc.gpsimd.collective_compute(
                        "AllGather",
                        mybir.AluOpType.bypass,
                        replica_groups=[list(range(NC))],
                        ins=[h_myB[hop].opt()],
                        outs=[h_fullB[hop].opt()],
                    )
            nc.sync.dma_start(out=out_d[:], in_=out_acc[:])
    nc.compile()
    return nc


def _prepare(x, eps, b1, b2, b3, si_list, ni_list, cfg):
    """Host-side folding + sharding. Returns (scheds, scalars, in_maps)."""
    N, NC, S, NB = cfg["N"], cfg["NCORES"], cfg["S"], cfg["NB"]
    SPAD, NPAD, _ = _derived(cfg)
    MW = 16

    scalars = dict(
        eps=float(eps),
        a1=[float((1.0 + b1[h]) * (1.0 + b3[h])) for h in range(HOPS)],
        c1=[float(1.0 + b3[h]) for h in range(HOPS)],
    )

    # padded-coordinate gather table of x (bf16), shared by all cores.
    # Row space is [A halves of all cores | B halves of all cores] so the
    # two split AllGathers write contiguous regions.
    HALF = (NB // 2) * P
    BHALF = SPAD - HALF

    def _pad_row(n):
        c, l = n // S, n % S
        return np.where(l < HALF, c * HALF + l,
                        NC * HALF + c * BHALF + (l - HALF))

    xg = np.zeros((NPAD, D), ml_dtypes.bfloat16)
    for c in range(NC):
        lo, hi = c * S, min((c + 1) * S, N)
        nloc = hi - lo
        na = min(nloc, HALF)
        xg[c * HALF: c * HALF + na] = x[lo:lo + na]
        if nloc > HALF:
            xg[NC * HALF + c * BHALF: NC * HALF + c * BHALF + (nloc - HALF)] = \
                x[lo + HALF:hi]

    iota = np.tile(np.arange(P, dtype=np.float32), (P, 1)).astype(ml_dtypes.bfloat16)
    iota = np.tile(iota[:, None, :], (1, MW, 1))

    # host-folded per-hop elementwise terms: wx1 = w (.) x + c1, and the
    # out accumulator's initial value (1+eps) x  (block-major [P, NB*D] layout)
    def _blockmajor(a):  # [SPAD, D] -> [P, NB*D]
        return np.ascontiguousarray(
            a.reshape(NB, P, D).transpose(1, 0, 2).reshape(P, NB * D))

    wx1_list = [[] for _ in range(HOPS)]
    oini_list = []
    for h in range(HOPS):
        deg = np.bincount(ni_list[h], minlength=N).astype(np.float32)
        untouched = (deg == 0).astype(np.float32)
        w = (1.0 + float(b3[h])) * (deg + untouched + float(b2[h]))
        c1 = np.float32(scalars["c1"][h])
        for c in range(NC):
            lo, hi = c * S, min((c + 1) * S, N)
            v = np.zeros((SPAD, D), np.float32)
            v[: hi - lo] = w[lo:hi, None] * x[lo:hi] + c1
            wx1_list[h].append(_blockmajor(v).astype(ml_dtypes.bfloat16))
    for c in range(NC):
        lo, hi = c * S, min((c + 1) * S, N)
        v = np.zeros((SPAD, D), np.float32)
        v[: hi - lo] = (1.0 + scalars["eps"]) * x[lo:hi]
        oini_list.append(_blockmajor(v).astype(ml_dtypes.bfloat16))

    scheds, idx_np, off_np = [], [], []
    for h in range(HOPS):
        si, ni = si_list[h], ni_list[h]
        si_pad = _pad_row(si)
        ni_core = ni // S
        ni_locs, si_pads = [], []
        for c in range(NC):
            m = ni_core == c
            ni_locs.append((ni[m] - c * S).astype(np.int64))
            si_pads.append(si_pad[m].astype(np.int64))
        sched, percore = _plan_hop(ni_locs, si_pads, cfg)
        scheds.append(sched)
        idx_np.append([pc["idx"] for pc in percore])
        off_np.append([pc["off"] for pc in percore])

    in_maps = []
    for c in range(NC):
        m = dict(xg=xg, oini=oini_list[c], iota=iota)
        for h in range(HOPS):
            m[f"idx{h}"] = idx_np[h][c]
            m[f"off{h}"] = off_np[h][c]
            m[f"wx1_{h}"] = wx1_list[h][c]
        in_maps.append(m)
    return scheds, scalars, in_maps


def run(x, eps, b1, b2, b3, si_list, ni_list, cfg, trace=False, hops=HOPS,
        **spmd_kwargs):
    scheds, scalars, in_maps = _prepare(x, eps, b1, b2, b3, si_list, ni_list, cfg)
    nc = _build_nc(cfg, scheds, scalars, hops=hops)
    res = run_bass_kernel_spmd(nc, in_maps, list(range(cfg["NCORES"])),
                               trace=trace, **spmd_kwargs)
    N, NC, S, NB = cfg["N"], cfg["NCORES"], cfg["S"], cfg["NB"]
    SPAD = NB * P
    parts = []
    for c in range(NC):
        o = res.results[c]["out"].astype(np.float32).reshape(
            P, NB, D).transpose(1, 0, 2).reshape(SPAD, D)
        lo, hi = c * S, min((c + 1) * S, N)
        parts.append(o[: hi - lo])
    return np.concatenate(parts, axis=0), res


def kernel(**inputs):
    x = np.asarray(inputs["x"], np.float32)
    eps = float(np.asarray(inputs["eps"]))
    b1 = np.asarray(inputs["beta1"], np.float32)
    b2 = np.asarray(inputs["beta2"], np.float32)
    b3 = np.asarray(inputs["beta3"], np.float32)
    si_list = [np.asarray(inputs[f"agg_scatter_index_{h}"]).astype(np.int64)
               for h in range(HOPS)]
    ni_list = [np.asarray(inputs[f"agg_node_index_{h}"]).astype(np.int64)
               for h in range(HOPS)]
    out, _ = run(x, eps, b1, b2, b3, si_list, ni_list, FULL_CFG)
    return out.astype(np.float32)
